# revision 1
# baseline (speedup 1.0000x reference)
"""Causal self-attention (B=8, T=1024, C=768, NH=12) on 8 TRN2 NeuronCores.

Strategy: pure batch data-parallel — core b computes batch element b end to
end (no collectives).

PE dtype rules learned from traces: fp32r matmuls run 1 cyc/row only with
K=128, M=128, N>=256 (2 cyc/row if K or M partial, 4 cyc/row if N<256);
bf16 always runs 1 cyc/row. So QKV / AV / proj run fp32r at full shape and
the K=64 QK matmuls + tiny mask matmuls run bf16.

Per-core dataflow (everything kept "transposed", i.e. [feature, time]):
  xT [C, T]                                  (host pre-transposes x[b])
  qkT[j, t] = Wqk[:, j].T x  (bf16)          attT-friendly layout
  v  [t, j] = x Wv           (fp32r)         AV-friendly layout, augmented
                                             with a ones column per head
  attT[tk, tq] = kT.T @ qT   per head pair   PSUM [128, 1024] tiles (two
                                             heads side by side, one exp each)
  causal mask: bf16 PE matmuls (ltri @ ident) add -1e30 into PSUM pre-exp
  expT = exp(scale * attT)   (no max-sub: |logits|<~3 for this problem)
  out_aug = [v | 1 | junk].T @ expT          M=128 keeps fp32r at 1 cyc/row;
                                             row 64 = softmax denominator,
                                             rows 65+ are junk (never read)
  rawT[j, t] = out_aug[d] * (1/denom)        denom broadcast via DMA
  yT[e, t] = Wp.T @ rawT + bp                output, host transposes back
"""

import os
import sys

import numpy as np

for _p in ("/opt/trn_rl_repo", "/root/.axon_site/_ro/trn_rl_repo"):
    if os.path.isdir(_p) and _p not in sys.path:
        sys.path.insert(0, _p)

import ml_dtypes

import concourse.bacc as bacc
import concourse.mybir as mybir
import concourse.tile as tile
from concourse.bass import ts
from concourse.bass_utils import run_bass_kernel_spmd

B, T, C = 8, 1024, 768
NH, HD = 12, 64
P = 128
NCORES = 8
CC = C // P            # 6 contraction chunks over C
JQK = 2 * C // P       # 12 output chunks for q|k
EC = C // P            # 6 output chunks for the projection
TQ = 512               # moving-dim tile (max psum bank width)
NTQ = T // TQ          # 2
NTK = T // P           # 8 key chunks
G = NH // 2            # 6 head pairs (two 64-wide heads per 128 partitions)
VW = 2 * HD + 2        # 130: per-pair v layout [d_even(64), 1, d_odd(64), 1]
VPAD = 63              # tail pad so the odd lhsT can always grab 128 cols
JV = 384               # v output tile width (3 head pairs)
SCALE = 1.0 / float(np.sqrt(HD))
F32 = mybir.dt.float32
F32R = mybir.dt.float32r
BF16 = mybir.dt.bfloat16
AF = mybir.ActivationFunctionType
ADD = mybir.AluOpType.add

_CACHE = {}


def _build():
    if "nc" in _CACHE:
        return _CACHE["nc"]

    nc = bacc.Bacc("TRN2", target_bir_lowering=False, debug=False)

    xT = nc.dram_tensor("xT", [C, T], F32R, kind="ExternalInput")
    wqk = nc.dram_tensor("wqk", [C, 2 * C], F32R, kind="ExternalInput")
    wv = nc.dram_tensor("wv", [C, C], F32R, kind="ExternalInput")
    wp = nc.dram_tensor("wp", [C, C], F32R, kind="ExternalInput")
    bqk = nc.dram_tensor("bqk", [P, JQK], F32, kind="ExternalInput")
    bvr = nc.dram_tensor("bvr", [P, C], F32, kind="ExternalInput")
    bp = nc.dram_tensor("bp", [P, EC], F32, kind="ExternalInput")
    ident = nc.dram_tensor("ident", [P, P], BF16, kind="ExternalInput")
    ltri = nc.dram_tensor("ltri", [P, P], BF16, kind="ExternalInput")
    mrow = nc.dram_tensor("mrow", [P, P], BF16, kind="ExternalInput")
    onesr = nc.dram_tensor("onesr", [P, 3 * P], BF16, kind="ExternalInput")
    yT = nc.dram_tensor("yT", [C, T], F32, kind="ExternalOutput")

    xT_r = xT[:].rearrange("(o p) t -> p o t", p=P)
    wqk_r = wqk[:].rearrange("(o p) j -> p o j", p=P)
    wv_r = wv[:].rearrange("(o p) j -> p o j", p=P)
    wp_r = wp[:].rearrange("(o p) e -> p o e", p=P)
    yT_r = yT[:].rearrange("(o p) t -> p o t", p=P)

    with tile.TileContext(nc) as tc:
        with (
            tc.tile_pool(name="const", bufs=1) as constp,
            tc.tile_pool(name="xt", bufs=6) as xtp,
            tc.tile_pool(name="wqk", bufs=4) as wqkp,
            tc.tile_pool(name="wv", bufs=1) as wvp,
            tc.tile_pool(name="qkt", bufs=1) as qkTp,
            tc.tile_pool(name="vaug", bufs=1) as vap,
            tc.tile_pool(name="raw", bufs=1) as rawp,
            tc.tile_pool(name="wp", bufs=6) as wpp,
            tc.tile_pool(name="exp", bufs=3) as expp,
            tc.tile_pool(name="rr", bufs=3) as rrp,
            tc.tile_pool(name="yt", bufs=3) as ytp,
            tc.tile_pool(name="psA", bufs=2, space="PSUM") as psA,
            tc.tile_pool(name="psB", bufs=4, space="PSUM") as psB,
        ):
            # ---- resident tensors -------------------------------------
            xts = []
            for cc in range(CC):
                xt_t = xtp.tile([P, T], F32R, tag="xt", name=f"xt{cc}")
                nc.sync.dma_start(xt_t[:], xT_r[:, cc, :])
                xts.append(xt_t)

            bqk_sb = constp.tile([P, JQK], F32)
            nc.sync.dma_start(bqk_sb[:], bqk[:])
            bv_sb = constp.tile([P, C], F32)
            nc.sync.dma_start(bv_sb[:], bvr[:])
            bp_sb = constp.tile([P, EC], F32)
            nc.sync.dma_start(bp_sb[:], bp[:])
            ident_sb = constp.tile([P, P], BF16)
            nc.sync.dma_start(ident_sb[:], ident[:])
            ltri_sb = constp.tile([P, P], BF16)
            nc.sync.dma_start(ltri_sb[:], ltri[:])
            mrow_sb = constp.tile([P, P], BF16)
            nc.sync.dma_start(mrow_sb[:], mrow[:])
            onesr_sb = constp.tile([P, 3 * P], BF16)
            nc.sync.dma_start(onesr_sb[:], onesr[:])

            qkT_sb = qkTp.tile([P, JQK, T], BF16)
            v_sb = vap.tile([P, NTK, G * VW + VPAD], BF16)
            v4 = v_sb[:, :, : G * VW].rearrange("p n (g w) -> p n g w", w=VW)
            rawT = rawp.tile([P, CC, T], F32R)

            # ---- v[t, j] = x Wv + b, interleaved per head pair --------
            # ones columns feed the softmax-denominator trick; tail pad is
            # junk-read by the odd head's M=128 lhsT (rows 65+ of its psum)
            onec = constp.tile([P, 1], F32)
            nc.vector.memset(onec[:], 1.0)
            zeroc = constp.tile([P, 1], F32)
            nc.vector.memset(zeroc[:], 0.0)
            ones_src = onec[:, None, None, :].to_broadcast([P, NTK, G, 1])
            nc.any.tensor_copy(v4[:, :, :, HD : HD + 1], ones_src)
            nc.any.tensor_copy(v4[:, :, :, VW - 1 : VW], ones_src)
            nc.any.tensor_copy(
                v_sb[:, :, G * VW :],
                zeroc[:, None, :].to_broadcast([P, NTK, VPAD]),
            )

            wv_sb = wvp.tile([P, CC, C], F32R)

            def v_phase():
                for tc_i in range(NTK):
                    for jn in range(C // JV):
                        ps = psB.tile([P, TQ], F32, tag="mm", name="psv")
                        for cc in range(CC):
                            nc.tensor.matmul(
                                ps[:, :JV],
                                xts[cc][:, ts(tc_i, P)],
                                wv_sb[:, cc, ts(jn, JV)],
                                start=(cc == 0),
                                stop=(cc == CC - 1),
                            )
                        g0 = jn * (JV // P)  # 3 head pairs per 384 cols
                        srcv = ps[:, :JV].rearrange(
                            "p (g h d) -> p g h d", h=2, d=HD
                        )
                        bias = bv_sb[:, ts(jn, JV)].rearrange(
                            "p (g h d) -> p g h d", h=2, d=HD
                        )
                        nc.vector.tensor_tensor(
                            v4[:, tc_i, g0 : g0 + 3, 0:HD],
                            srcv[:, :, 0, :],
                            bias[:, :, 0, :],
                            ADD,
                        )
                        nc.vector.tensor_tensor(
                            v4[:, tc_i, g0 : g0 + 3, HD + 1 : VW - 1],
                            srcv[:, :, 1, :],
                            bias[:, :, 1, :],
                            ADD,
                        )

            def load_wt(jc):
                wt = wqkp.tile([P, CC, P], F32R, tag="wqk", name="wt")
                nc.sync.dma_start(wt[:], wqk_r[:, :, ts(jc, P)])
                return wt

            def qkt_chunk(jc, wt):
                for t2 in range(NTQ):
                    ps = psB.tile([P, TQ], F32, tag="mm", name="psq")
                    for cc in range(CC):
                        nc.tensor.matmul(
                            ps[:],
                            wt[:, cc, :],
                            xts[cc][:, ts(t2, TQ)],
                            start=(cc == 0),
                            stop=(cc == CC - 1),
                        )
                    nc.vector.tensor_scalar_add(
                        qkT_sb[:, jc, ts(t2, TQ)],
                        ps[:],
                        bqk_sb[:, jc : jc + 1],
                    )

            def attn_block(g, t2):
                jq, jk = g, G + g
                hi = 4 * (t2 + 1)  # causal: key chunks 0..hi-1
                avs = []
                for par in (0, 1):
                    av = psB.tile([P, TQ], F32, tag="mm", name=f"av{par}")
                    avs.append(av)
                for tkc in range(hi):
                    csr = tkc * P - t2 * TQ  # diag block start col
                    cs = max(0, csr)
                    pa = psA.tile([P, 2 * TQ], F32, tag="pa", name="pa")
                    for par in (0, 1):
                        qrow = HD * par
                        off = par * TQ
                        nc.tensor.matmul(
                            pa[:, off + cs : off + TQ],
                            qkT_sb[qrow : qrow + HD, jk, ts(tkc, P)],
                            qkT_sb[
                                qrow : qrow + HD,
                                jq,
                                t2 * TQ + cs : (t2 + 1) * TQ,
                            ],
                            start=True,
                            stop=(csr < 0),
                        )
                        if csr >= 0:
                            # add -1e30 above the diagonal via the PE:
                            # pa[tk, off+cs+c] += ltri[c, tk]
                            nc.tensor.matmul(
                                pa[:, off + cs : off + cs + P],
                                ltri_sb[:],
                                ident_sb[:],
                                start=False,
                                stop=True,
                            )
                        if par == 1 and cs > 0:
                            # fill the unwritten gap between the two halves
                            # with -1e30 (exp -> 0, never read by AV) so one
                            # wide exp can span both heads
                            nc.tensor.matmul(
                                pa[:, TQ : TQ + cs],
                                mrow_sb[:],
                                onesr_sb[:, :cs],
                                start=True,
                                stop=True,
                            )
                    e = expp.tile([P, 2 * TQ], BF16, tag="exp", name="e")
                    nc.scalar.activation(
                        e[:, cs:], pa[:, cs:], AF.Exp, scale=SCALE
                    )
                    for par in (0, 1):
                        off = par * TQ
                        vlo = g * VW + (HD + 1) * par
                        nc.tensor.matmul(
                            avs[par][:, cs:],
                            v_sb[:, tkc, vlo : vlo + P],
                            e[:, off + cs : off + TQ],
                            start=(tkc == 0),
                            stop=(tkc == hi - 1),
                        )
                # evacuate out_aug to SBUF right away (frees the psum
                # banks), then denom -> recip -> broadcast -> scale.
                # DVE lanes can't cross partitions, so everything runs at
                # base 0; the odd head's result reaches partitions 64:128
                # of rawT via an SBUF->SBUF DMA.
                asbs = []
                for par in (0, 1):
                    asb = rrp.tile([65, TQ], F32, tag="avsb", name=f"asb{par}")
                    nc.vector.tensor_scalar_add(asb[:], avs[par][0:65, :], 0.0)
                    asbs.append(asb)
                # single-lane reciprocal of [1, 512] costs ~3.3us on the
                # DVE; reshape the denominators across all 128 lanes via DMA
                # instead (recip then costs ~0.1us). Stages are interleaved
                # across the two parities so their DMA hops overlap.
                rds, rd2s, rros, rrs = [], [], [], []
                for par in (0, 1):
                    rd = rrp.tile([P, 4], F32, tag="rd", name=f"rd{par}")
                    nc.sync.dma_start(rd[:], asbs[par][64:65, :])
                    rds.append(rd)
                for par in (0, 1):
                    rd2 = rrp.tile([P, 4], F32, tag="rd2", name=f"rd2{par}")
                    nc.vector.reciprocal(rd2[:], rds[par][:])
                    rd2s.append(rd2)
                for par in (0, 1):
                    rro = rrp.tile([1, TQ], F32, tag="rro", name=f"rro{par}")
                    nc.sync.dma_start(rro[0:1, :], rd2s[par][:])
                    rros.append(rro)
                for par in (0, 1):
                    rr = rrp.tile([64, TQ], F32, tag="rr", name=f"rr{par}")
                    nc.sync.dma_start(
                        rr[:],
                        rros[par][0:1, None, :].to_broadcast([1, 64, TQ]),
                    )
                    rrs.append(rr)
                nc.vector.tensor_mul(
                    rawT[0:64, g, ts(t2, TQ)], asbs[0][0:64, :], rrs[0][:]
                )
                tmp = rrp.tile([64, TQ], F32R, tag="otmp", name="tmp")
                nc.vector.tensor_mul(tmp[:], asbs[1][0:64, :], rrs[1][:])
                nc.sync.dma_start(rawT[64:128, g, ts(t2, TQ)], tmp[:])

            def load_wpt(ec):
                wpt = wpp.tile([P, CC, P], F32R, tag="wp", name="wpt")
                nc.sync.dma_start(wpt[:], wp_r[:, :, ts(ec, P)])
                return wpt

            def proj_half(t2, wpts):
                for ec in range(EC):
                    wpt = wpts[ec]
                    ps = psB.tile([P, TQ], F32, tag="mm", name="psp_")
                    for jc in range(CC):
                        nc.tensor.matmul(
                            ps[:],
                            wpt[:, jc, :],
                            rawT[:, jc, ts(t2, TQ)],
                            start=(jc == 0),
                            stop=(jc == CC - 1),
                        )
                    yt = ytp.tile([P, TQ], F32, tag="yt", name="yt")
                    nc.scalar.activation(
                        yt[:], ps[:], AF.Identity, bias=bp_sb[:, ec : ec + 1]
                    )
                    nc.sync.dma_start(yT_r[:, ec, ts(t2, TQ)], yt[:])

            # emission order: start the PE on qkT(g=0) while weights
            # stream in, fill with v, sweep attention t2=0, project the
            # first output half while attention t2=1 runs, project rest
            wts = (load_wt(0), load_wt(G))
            qkt_chunk(0, wts[0])
            qkt_chunk(G, wts[1])
            nc.sync.dma_start(wv_sb[:], wv_r)
            v_phase()
            nxt = (load_wt(1), load_wt(G + 1))
            attn_block(0, 0)
            wpts = []
            for g in range(1, G):
                wts = nxt
                qkt_chunk(g, wts[0])
                qkt_chunk(G + g, wts[1])
                if g < G - 1:
                    nxt = (load_wt(g + 1), load_wt(G + g + 1))
                else:
                    wpts = [load_wpt(ec) for ec in range(EC)]
                attn_block(g, 0)
            proj_half(0, wpts)
            for g in range(G):
                attn_block(g, 1)
            proj_half(1, wpts)

    nc.compile()
    _CACHE["nc"] = nc
    return nc


def _round_fp32r(a):
    """Round fp32 to fp32r (11-bit mantissa) the way the PE expects."""
    u = np.ascontiguousarray(a, dtype=np.float32).view(np.uint32)
    u = ((u.astype(np.uint64) + 0x800) & 0xFFFFF000).astype(np.uint32)
    return u.view(np.float32)


def make_in_maps(x, w_attn, b_attn, w_proj, b_proj):
    x = np.ascontiguousarray(np.asarray(x, dtype=np.float32))
    w_attn = np.ascontiguousarray(np.asarray(w_attn, dtype=np.float32))
    b_attn = np.ascontiguousarray(np.asarray(b_attn, dtype=np.float32))
    w_proj = np.ascontiguousarray(np.asarray(w_proj, dtype=np.float32))
    b_proj = np.ascontiguousarray(np.asarray(b_proj, dtype=np.float32))

    bf = ml_dtypes.bfloat16
    wqk = _round_fp32r(w_attn[:, : 2 * C])
    wv = _round_fp32r(w_attn[:, 2 * C :])
    w_proj_r = _round_fp32r(w_proj)
    bqk = np.ascontiguousarray(b_attn[: 2 * C].reshape(JQK, P).T)
    bvr = np.ascontiguousarray(np.tile(b_attn[2 * C :][None, :], (P, 1)))
    bp = np.ascontiguousarray(b_proj.reshape(EC, P).T)
    ident = np.eye(P, dtype=bf)
    # ltri[c, tk] = -1e30 where c < tk (masks tq_local < tk_local)
    ltri = np.where(
        np.arange(P)[:, None] < np.arange(P)[None, :], -1e30, 0.0
    ).astype(bf)
    mrow = np.zeros((P, P), dtype=bf)
    mrow[0, :] = bf(-1e30)
    onesr = np.zeros((P, 3 * P), dtype=bf)
    onesr[0, :] = bf(1.0)

    shared = {
        "wqk": wqk,
        "wv": wv,
        "wp": w_proj_r,
        "bqk": bqk,
        "bvr": bvr,
        "bp": bp,
        "ident": ident,
        "ltri": ltri,
        "mrow": mrow,
        "onesr": onesr,
    }
    return [
        {"xT": _round_fp32r(x[b].T), **shared} for b in range(NCORES)
    ]


def kernel(**inputs):
    nc = _build()
    in_maps = make_in_maps(
        inputs["x"],
        inputs["w_attn"],
        inputs["b_attn"],
        inputs["w_proj"],
        inputs["b_proj"],
    )
    res = run_bass_kernel_spmd(nc, in_maps, list(range(NCORES)))
    out = np.stack(
        [np.ascontiguousarray(res.results[b]["yT"].T) for b in range(NCORES)]
    )
    return out.astype(np.float32)



# revision 2
# speedup vs baseline: 1.1009x; 1.1009x over previous
"""Causal self-attention (B=8, T=1024, C=768, NH=12) on 8 TRN2 NeuronCores.

Strategy: pure batch data-parallel — core b computes batch element b end to
end (no collectives).

PE cost model (instruction_cost_v2): matmul streaming cost = N output
cols x 0.417ns; K<=128 is the parallel dim.  K=64 matmuls with lhsT at
partition 0 vs 64 get tile_position (0,0)/(64,0) automatically and run
CONCURRENTLY (2x row tiling) — but only if nothing in between switches
the array back to 128-row mode (mode switch = drain).

Per-core dataflow (everything kept "transposed", i.e. [feature, time]):
  xT [C, T]                                  (host pre-transposes x[b])
  qkT[j, t] = Wqk[:, j].T x  (bf16)          psum evac on Act (+bias)
  v  [t, j] = x Wv           (fp32r)         AV-friendly layout, augmented
                                             with a ones column per head
  attT[tk, tq] = kT.T @ qT   per head pair   K=64 pair-tiled matmuls; the
                                             two parities land at psum cols
                                             [cs,TQ) and [TQ,2TQ-cs) (par1
                                             shifted left by cs: no gap)
  causal diag mask: two K=64 triangular bf16 matmuls per (diag,par) add
                                             -1e30 in the SAME 64-row PE
                                             mode (no array mode switch)
  expT = exp(scale * attT)   one Act op over the contiguous [cs, 2TQ-cs)
  out_aug = [v | 1 | junk].T @ expT          row 64 = softmax denominator
  rawT[j, t] = out_aug[d] * (1/denom)        denom broadcast via DMA
  yT[e, t] = Wp.T @ rawT + bp'               bp' = Wp.T bv + bp (host),
                                             folding the v bias for free
"""

import os
import sys

import numpy as np

for _p in ("/opt/trn_rl_repo", "/root/.axon_site/_ro/trn_rl_repo"):
    if os.path.isdir(_p) and _p not in sys.path:
        sys.path.insert(0, _p)

import ml_dtypes

import concourse.bacc as bacc
import concourse.mybir as mybir
import concourse.tile as tile
from concourse.bass import ts
from concourse.bass_utils import run_bass_kernel_spmd

B, T, C = 8, 1024, 768
NH, HD = 12, 64
P = 128
NCORES = 8
CC = C // P            # 6 contraction chunks over C
JQK = 2 * C // P       # 12 output chunks for q|k
EC = C // P            # 6 output chunks for the projection
TQ = 512               # moving-dim tile (max psum bank width)
NTQ = T // TQ          # 2
NTK = T // P           # 8 key chunks
G = NH // 2            # 6 head pairs (two 64-wide heads per 128 partitions)
VW = 2 * HD + 2        # 130: per-pair v layout [d_even(64), 1, d_odd(64), 1]
VPAD = 63              # tail pad so the odd lhsT can always grab 128 cols
JV = 384               # v output tile width (3 head pairs)
SCALE = 1.0 / float(np.sqrt(HD))
F32 = mybir.dt.float32
F32R = mybir.dt.float32r
BF16 = mybir.dt.bfloat16
AF = mybir.ActivationFunctionType
ADD = mybir.AluOpType.add

_CACHE = {}


def _build():
    if "nc" in _CACHE:
        return _CACHE["nc"]

    nc = bacc.Bacc("TRN2", target_bir_lowering=False, debug=False)

    xT = nc.dram_tensor("xT", [C, T], F32R, kind="ExternalInput")
    wqk = nc.dram_tensor("wqk", [C, 2 * C], F32R, kind="ExternalInput")
    wv = nc.dram_tensor("wv", [C, C], F32R, kind="ExternalInput")
    wp = nc.dram_tensor("wp", [C, C], F32R, kind="ExternalInput")
    bqk = nc.dram_tensor("bqk", [P, JQK], F32, kind="ExternalInput")
    bp = nc.dram_tensor("bp", [P, EC], F32, kind="ExternalInput")
    tri = nc.dram_tensor("tri", [P, 2, P], BF16, kind="ExternalInput")
    id64 = nc.dram_tensor("id64", [P, HD], BF16, kind="ExternalInput")
    yT = nc.dram_tensor("yT", [C, T], F32, kind="ExternalOutput")

    xT_r = xT[:].rearrange("(o p) t -> p o t", p=P)
    wqk_r = wqk[:].rearrange("(o p) j -> p o j", p=P)
    wv_r = wv[:].rearrange("(o p) j -> p o j", p=P)
    wp_r = wp[:].rearrange("(o p) e -> p o e", p=P)
    yT_r = yT[:].rearrange("(o p) t -> p o t", p=P)

    with tile.TileContext(nc) as tc:
        with (
            tc.tile_pool(name="const", bufs=1) as constp,
            tc.tile_pool(name="xt", bufs=6) as xtp,
            tc.tile_pool(name="wqk", bufs=4) as wqkp,
            tc.tile_pool(name="wv", bufs=1) as wvp,
            tc.tile_pool(name="qkt", bufs=1) as qkTp,
            tc.tile_pool(name="vaug", bufs=1) as vap,
            tc.tile_pool(name="raw", bufs=1) as rawp,
            tc.tile_pool(name="wp", bufs=6) as wpp,
            tc.tile_pool(name="exp", bufs=3) as expp,
            tc.tile_pool(name="rr", bufs=3) as rrp,
            tc.tile_pool(name="yt", bufs=3) as ytp,
            tc.tile_pool(name="psA", bufs=2, space="PSUM") as psA,
            tc.tile_pool(name="psB", bufs=2, space="PSUM") as psB,
        ):
            # ---- resident tensors -------------------------------------
            xts = []
            for cc in range(CC):
                xt_t = xtp.tile([P, T], F32R, tag="xt", name=f"xt{cc}")
                nc.sync.dma_start(xt_t[:], xT_r[:, cc, :])
                xts.append(xt_t)

            bqk_sb = constp.tile([P, JQK], F32)
            nc.sync.dma_start(bqk_sb[:], bqk[:])
            bp_sb = constp.tile([P, EC], F32)
            nc.sync.dma_start(bp_sb[:], bp[:])
            tri_sb = constp.tile([P, 2, P], BF16)
            nc.sync.dma_start(tri_sb[:], tri[:])
            id64_sb = constp.tile([P, HD], BF16)
            nc.sync.dma_start(id64_sb[:], id64[:])

            qkT_sb = qkTp.tile([P, JQK, T], BF16)
            v_sb = vap.tile([P, NTK, G * VW + VPAD], BF16)
            v4 = v_sb[:, :, : G * VW].rearrange("p n (g w) -> p n g w", w=VW)
            rawT = rawp.tile([P, CC, T], F32R)

            # ---- v[t, j] = x Wv, interleaved per head pair ------------
            # ones columns feed the softmax-denominator trick; tail pad is
            # junk-read by the odd head's M=128 lhsT (rows 65+ of its psum)
            onec = constp.tile([P, 1], F32)
            nc.vector.memset(onec[:], 1.0)
            zeroc = constp.tile([P, 1], F32)
            nc.vector.memset(zeroc[:], 0.0)
            ones_src = onec[:, None, None, :].to_broadcast([P, NTK, G, 1])
            nc.any.tensor_copy(v4[:, :, :, HD : HD + 1], ones_src)
            nc.any.tensor_copy(v4[:, :, :, VW - 1 : VW], ones_src)
            nc.any.tensor_copy(
                v_sb[:, :, G * VW :],
                zeroc[:, None, :].to_broadcast([P, NTK, VPAD]),
            )

            wv_sb = wvp.tile([P, CC, C], F32R)

            def v_part(tcs):
                for tc_i in tcs:
                    for jn in range(C // JV):
                        ps = psB.tile([P, TQ], F32, tag="mm", name="psv")
                        for cc in range(CC):
                            nc.tensor.matmul(
                                ps[:, :JV],
                                xts[cc][:, ts(tc_i, P)],
                                wv_sb[:, cc, ts(jn, JV)],
                                start=(cc == 0),
                                stop=(cc == CC - 1),
                            )
                        g0 = jn * (JV // P)  # 3 head pairs per 384 cols
                        srcv = ps[:, :JV].rearrange(
                            "p (g h d) -> p g h d", h=2, d=HD
                        )
                        # psum -> interleaved sbuf layout on the Act engine
                        # (DVE stays free for the divide chains)
                        nc.scalar.activation(
                            v4[:, tc_i, g0 : g0 + 3, 0:HD],
                            srcv[:, :, 0, :],
                            AF.Identity,
                        )
                        nc.scalar.activation(
                            v4[:, tc_i, g0 : g0 + 3, HD + 1 : VW - 1],
                            srcv[:, :, 1, :],
                            AF.Identity,
                        )

            def load_wt(jc):
                wt = wqkp.tile([P, CC, P], F32R, tag="wqk", name="wt")
                nc.sync.dma_start(wt[:], wqk_r[:, :, ts(jc, P)])
                return wt

            def qkt_chunk(jc, wt):
                for t2 in range(NTQ):
                    ps = psB.tile([P, TQ], F32, tag="mm", name="psq")
                    for cc in range(CC):
                        nc.tensor.matmul(
                            ps[:],
                            wt[:, cc, :],
                            xts[cc][:, ts(t2, TQ)],
                            start=(cc == 0),
                            stop=(cc == CC - 1),
                        )
                    nc.scalar.activation(
                        qkT_sb[:, jc, ts(t2, TQ)],
                        ps[:],
                        AF.Identity,
                        bias=bqk_sb[:, jc : jc + 1],
                    )

            def attn_block(g, t2):
                jq, jk = g, G + g
                hi = 4 * (t2 + 1)  # causal: key chunks 0..hi-1
                avs = []
                for par in (0, 1):
                    av = psB.tile([P, TQ], F32, tag="av", name=f"av{par}")
                    avs.append(av)
                for g0 in range(0, hi, 2):
                    grp = [tkc for tkc in (g0, g0 + 1) if tkc < hi]
                    pas, css = {}, {}
                    # all QK + diag-mask matmuls of the group first: they
                    # are K=64 and pair-tile as (0,0)/(64,0); keeping them
                    # adjacent avoids PE array mode switches
                    for tkc in grp:
                        csr = tkc * P - t2 * TQ  # diag block start col
                        cs = max(0, csr)
                        pa = psA.tile([P, 2 * TQ], F32, tag="pa", name="pa")
                        pas[tkc], css[tkc] = pa, cs
                        for par in (0, 1):
                            qrow = HD * par
                            lo = cs if par == 0 else TQ
                            nc.tensor.matmul(
                                pa[:, lo : lo + TQ - cs],
                                qkT_sb[qrow : qrow + HD, jk, ts(tkc, P)],
                                qkT_sb[
                                    qrow : qrow + HD,
                                    jq,
                                    t2 * TQ + cs : (t2 + 1) * TQ,
                                ],
                                start=True,
                                stop=(csr < 0),
                            )
                        if csr >= 0:
                            # add -1e30 above the diagonal with two K=64
                            # triangular matmuls (same row-tiled mode):
                            # pa[tk, lo+j] += tri[j, tk]
                            for par in (0, 1):
                                qrow = HD * par
                                lo = cs if par == 0 else TQ
                                for half in (0, 1):
                                    nc.tensor.matmul(
                                        pa[
                                            :,
                                            lo + HD * half : lo + HD * (half + 1),
                                        ],
                                        tri_sb[qrow : qrow + HD, half, :],
                                        id64_sb[qrow : qrow + HD, :],
                                        start=False,
                                        stop=(half == 1),
                                    )
                    for tkc in grp:
                        pa, cs = pas[tkc], css[tkc]
                        e = expp.tile([P, 2 * TQ], BF16, tag="exp", name="e")
                        nc.scalar.activation(
                            e[:, cs : 2 * TQ - cs],
                            pa[:, cs : 2 * TQ - cs],
                            AF.Exp,
                            scale=SCALE,
                        )
                        for par in (0, 1):
                            lo = cs if par == 0 else TQ
                            vlo = g * VW + (HD + 1) * par
                            nc.tensor.matmul(
                                avs[par][:, cs:],
                                v_sb[:, tkc, vlo : vlo + P],
                                e[:, lo : lo + TQ - cs],
                                start=(tkc == 0),
                                stop=(tkc == hi - 1),
                            )
                # evacuate out_aug to SBUF right away (frees the psum
                # banks), then denom -> recip -> broadcast -> scale.
                # DVE lanes can't cross partitions, so everything runs at
                # base 0; the odd head's result reaches partitions 64:128
                # of rawT via an SBUF->SBUF DMA.
                asbs = []
                for par in (0, 1):
                    asb = rrp.tile([65, TQ], F32, tag="avsb", name=f"asb{par}")
                    nc.vector.tensor_scalar_add(asb[:], avs[par][0:65, :], 0.0)
                    asbs.append(asb)
                # single-lane reciprocal of [1, 512] costs ~3.3us on the
                # DVE; reshape the denominators across all 128 lanes via DMA
                # instead (recip then costs ~0.1us). Stages are interleaved
                # across the two parities so their DMA hops overlap.
                rds, rd2s, rros, rrs = [], [], [], []
                for par in (0, 1):
                    rd = rrp.tile([P, 4], F32, tag="rd", name=f"rd{par}")
                    nc.sync.dma_start(rd[:], asbs[par][64:65, :])
                    rds.append(rd)
                for par in (0, 1):
                    rd2 = rrp.tile([P, 4], F32, tag="rd2", name=f"rd2{par}")
                    nc.vector.reciprocal(rd2[:], rds[par][:])
                    rd2s.append(rd2)
                for par in (0, 1):
                    rro = rrp.tile([1, TQ], F32, tag="rro", name=f"rro{par}")
                    nc.sync.dma_start(rro[0:1, :], rd2s[par][:])
                    rros.append(rro)
                for par in (0, 1):
                    rr = rrp.tile([64, TQ], F32, tag="rr", name=f"rr{par}")
                    nc.sync.dma_start(
                        rr[:],
                        rros[par][0:1, None, :].to_broadcast([1, 64, TQ]),
                    )
                    rrs.append(rr)
                nc.vector.tensor_mul(
                    rawT[0:64, g, ts(t2, TQ)], asbs[0][0:64, :], rrs[0][:]
                )
                tmp = rrp.tile([64, TQ], F32R, tag="otmp", name="tmp")
                nc.vector.tensor_mul(tmp[:], asbs[1][0:64, :], rrs[1][:])
                nc.sync.dma_start(rawT[64:128, g, ts(t2, TQ)], tmp[:])

            def load_wpt(ec):
                wpt = wpp.tile([P, CC, P], F32R, tag="wp", name="wpt")
                nc.sync.dma_start(wpt[:], wp_r[:, :, ts(ec, P)])
                return wpt

            def proj_unit(t2, ec, wpt):
                ps = psB.tile([P, TQ], F32, tag="mm", name="psp_")
                for jc in range(CC):
                    nc.tensor.matmul(
                        ps[:],
                        wpt[:, jc, :],
                        rawT[:, jc, ts(t2, TQ)],
                        start=(jc == 0),
                        stop=(jc == CC - 1),
                    )
                yt = ytp.tile([P, TQ], F32, tag="yt", name="yt")
                nc.vector.tensor_scalar_add(yt[:], ps[:], bp_sb[:, ec : ec + 1])
                nc.sync.dma_start(yT_r[:, ec, ts(t2, TQ)], yt[:])

            # emission order: start the PE on qkT(g=0) while weights
            # stream in, fill v as needed (tkc 0-3 for the t2=0 sweep,
            # rest spread), run attention t2=0 interleaved with the next
            # pair's qkT, then attention t2=1 with proj(t2=0) chunks
            # interleaved, then the proj(t2=1) tail.
            wts = (load_wt(0), load_wt(G))
            qkt_chunk(0, wts[0])
            qkt_chunk(G, wts[1])
            nc.sync.dma_start(wv_sb[:], wv_r)
            v_part(range(0, 4))
            nxt = (load_wt(1), load_wt(G + 1))
            attn_block(0, 0)
            v_part(range(4, 6))
            wpts = []
            for g in range(1, G):
                wts = nxt
                qkt_chunk(g, wts[0])
                qkt_chunk(G + g, wts[1])
                if g < G - 1:
                    nxt = (load_wt(g + 1), load_wt(G + g + 1))
                else:
                    wpts = [load_wpt(ec) for ec in range(EC)]
                if g == 1:
                    v_part(range(6, 8))
                attn_block(g, 0)
            for g in range(G):
                attn_block(g, 1)
                proj_unit(0, g, wpts[g])
            for ec in range(EC):
                proj_unit(1, ec, wpts[ec])

    nc.compile()
    _CACHE["nc"] = nc
    return nc


def _round_fp32r(a):
    """Round fp32 to fp32r (11-bit mantissa) the way the PE expects."""
    u = np.ascontiguousarray(a, dtype=np.float32).view(np.uint32)
    u = ((u.astype(np.uint64) + 0x800) & 0xFFFFF000).astype(np.uint32)
    return u.view(np.float32)


def make_in_maps(x, w_attn, b_attn, w_proj, b_proj):
    x = np.ascontiguousarray(np.asarray(x, dtype=np.float32))
    w_attn = np.ascontiguousarray(np.asarray(w_attn, dtype=np.float32))
    b_attn = np.ascontiguousarray(np.asarray(b_attn, dtype=np.float32))
    w_proj = np.ascontiguousarray(np.asarray(w_proj, dtype=np.float32))
    b_proj = np.ascontiguousarray(np.asarray(b_proj, dtype=np.float32))

    bf = ml_dtypes.bfloat16
    wqk = _round_fp32r(w_attn[:, : 2 * C])
    wv = _round_fp32r(w_attn[:, 2 * C :])
    w_proj_r = _round_fp32r(w_proj)
    bqk = np.ascontiguousarray(b_attn[: 2 * C].reshape(JQK, P).T)
    # the v bias folds into the projection bias: y = Wp.T (raw + bv) + bp
    bv = b_attn[2 * C :].astype(np.float64)
    bp_eff = (w_proj.astype(np.float64).T @ bv + b_proj).astype(np.float32)
    bp = np.ascontiguousarray(bp_eff.reshape(EC, P).T)
    # tri[qrow+r, 0, tk] masks j=r    < tk; tri[qrow+r, 1, tk] masks 64+r < tk
    tri = np.zeros((P, 2, P), dtype=bf)
    tk = np.arange(P)[None, :]
    for qrow in (0, HD):
        r = np.arange(HD)[:, None]
        tri[qrow : qrow + HD, 0, :] = np.where(r < tk, -1e30, 0.0).astype(bf)
        tri[qrow : qrow + HD, 1, :] = np.where(r + HD < tk, -1e30, 0.0).astype(
            bf
        )
    id64 = np.zeros((P, HD), dtype=bf)
    for qrow in (0, HD):
        id64[qrow : qrow + HD, :] = np.eye(HD, dtype=bf)

    shared = {
        "wqk": wqk,
        "wv": wv,
        "wp": w_proj_r,
        "bqk": bqk,
        "bp": bp,
        "tri": tri,
        "id64": id64,
    }
    return [
        {"xT": _round_fp32r(x[b].T), **shared} for b in range(NCORES)
    ]


def kernel(**inputs):
    nc = _build()
    in_maps = make_in_maps(
        inputs["x"],
        inputs["w_attn"],
        inputs["b_attn"],
        inputs["w_proj"],
        inputs["b_proj"],
    )
    res = run_bass_kernel_spmd(nc, in_maps, list(range(NCORES)))
    out = np.stack(
        [np.ascontiguousarray(res.results[b]["yT"].T) for b in range(NCORES)]
    )
    return out.astype(np.float32)


# revision 5
# speedup vs baseline: 1.3967x; 1.2688x over previous
"""Causal self-attention (B=8, T=1024, C=768, NH=12) on 8 TRN2 NeuronCores.

Strategy: pure batch data-parallel — core b computes batch element b end to
end (no collectives).

PE cost model (instruction_cost_v2): matmul streaming cost = N output
cols x 0.417ns; K<=128 is the parallel dim.  K=64 matmuls with lhsT at
partition 0 vs 64 get tile_position (0,0)/(64,0) automatically and run
CONCURRENTLY (2x row tiling) — but only if nothing in between switches
the array back to 128-row mode (mode switch = drain).

Engines are strict-FIFO, so a long-latency wait at the head of a queue
(e.g. a DVE multiply waiting on a DMA round-trip) blocks everything
behind it.  The softmax divide chain is therefore DEFERRED by one
attention block: block k emits only psum->sbuf evac + the denominator
gather DMA; the recip/broadcast/multiply for block k are emitted after
block k+1's evac, by which point their DMA inputs have landed.

Per-core dataflow (everything kept "transposed", i.e. [feature, time]):
  xT [C, T]                                  (host pre-transposes x[b])
  qkT[j, t] = Wqk[:, j].T x  (bf16)          psum evac on Act (+bias)
  v  [t, j] = x Wv           (fp32r)         AV-friendly layout, augmented
                                             with a ones column per head
  attT[tk, tq] = kT.T @ qT   per head pair   K=64 pair-tiled matmuls; the
                                             two parities land at psum cols
                                             [cs,TQ) and [TQ,2TQ-cs) (par1
                                             shifted left by cs: no gap)
  causal diag mask: two K=64 triangular bf16 matmuls per (diag,par) add
                                             -1e30 in the SAME 64-row mode
  expT = exp(scale * attT)   one Act op over the contiguous [cs, 2TQ-cs)
  out_aug = [v | 1 | junk].T @ expT          row 64 = softmax denominator
  rawT[j, t] = out_aug[d] * (1/denom)        denom broadcast via DMA (bf16)
  yT[e, t] = Wp.T @ rawT + bp'               bp' = Wp.T bv + bp (host).
                                             The t2=1 half is split into a
                                             5-chunk partial + a final
                                             single matmul so the tail
                                             doesn't serialize behind the
                                             last divide chain.
"""

import os
import sys

import numpy as np

for _p in ("/opt/trn_rl_repo", "/root/.axon_site/_ro/trn_rl_repo"):
    if os.path.isdir(_p) and _p not in sys.path:
        sys.path.insert(0, _p)

import ml_dtypes

import concourse.bacc as bacc
import concourse.mybir as mybir
import concourse.tile as tile
from concourse.bass import ts
from concourse.bass_utils import run_bass_kernel_spmd

B, T, C = 8, 1024, 768
NH, HD = 12, 64
P = 128
NCORES = 8
CC = C // P            # 6 contraction chunks over C
JQK = 2 * C // P       # 12 output chunks for q|k
EC = C // P            # 6 output chunks for the projection
TQ = 512               # moving-dim tile (max psum bank width)
NTQ = T // TQ          # 2
NTK = T // P           # 8 key chunks
G = NH // 2            # 6 head pairs (two 64-wide heads per 128 partitions)
VW = 2 * HD + 2        # 130: per-pair v layout [d_even(64), 1, d_odd(64), 1]
VPAD = 63              # tail pad so the odd lhsT can always grab 128 cols
JV = 384               # v output tile width (3 head pairs)
SCALE = 1.0 / float(np.sqrt(HD))
F32 = mybir.dt.float32
F32R = mybir.dt.float32r
BF16 = mybir.dt.bfloat16
AF = mybir.ActivationFunctionType
ADD = mybir.AluOpType.add
MUL = mybir.AluOpType.mult

_CACHE = {}


def _build():
    if "nc" in _CACHE:
        return _CACHE["nc"]

    nc = bacc.Bacc("TRN2", target_bir_lowering=False, debug=False)

    xT = nc.dram_tensor("xT", [C, T], F32R, kind="ExternalInput")
    wqk = nc.dram_tensor("wqk", [C, 2 * C], F32R, kind="ExternalInput")
    wv = nc.dram_tensor("wv", [C, C], F32R, kind="ExternalInput")
    wp = nc.dram_tensor("wp", [C, C], F32R, kind="ExternalInput")
    bqk = nc.dram_tensor("bqk", [P, JQK], F32, kind="ExternalInput")
    bp = nc.dram_tensor("bp", [P, EC], F32, kind="ExternalInput")
    tri = nc.dram_tensor("tri", [P, 2, P], BF16, kind="ExternalInput")
    id64 = nc.dram_tensor("id64", [P, HD], BF16, kind="ExternalInput")
    yT = nc.dram_tensor("yT", [C, T], F32, kind="ExternalOutput")

    xT_r = xT[:].rearrange("(o p) t -> p o t", p=P)
    wqk_r = wqk[:].rearrange("(o p) j -> p o j", p=P)
    wv_r = wv[:].rearrange("(o p) j -> p o j", p=P)
    wp_r = wp[:].rearrange("(o p) e -> p o e", p=P)
    yT_r = yT[:].rearrange("(o p) t -> p o t", p=P)

    with tile.TileContext(nc) as tc:
        with (
            tc.tile_pool(name="const", bufs=1) as constp,
            tc.tile_pool(name="xt", bufs=6) as xtp,
            tc.tile_pool(name="wqk", bufs=4) as wqkp,
            tc.tile_pool(name="wv", bufs=1) as wvp,
            tc.tile_pool(name="qkt", bufs=1) as qkTp,
            tc.tile_pool(name="vaug", bufs=1) as vap,
            tc.tile_pool(name="raw", bufs=1) as rawp,
            tc.tile_pool(name="wp", bufs=6) as wpp,
            tc.tile_pool(name="exp", bufs=3) as expp,
            tc.tile_pool(name="rr", bufs=3) as rrp,
            tc.tile_pool(name="yt", bufs=3) as ytp,
            tc.tile_pool(name="yta", bufs=6) as ytap,
            tc.tile_pool(name="psA", bufs=2, space="PSUM") as psA,
            tc.tile_pool(name="psB", bufs=2, space="PSUM") as psB,
        ):
            # ---- resident tensors (DMA issue order matters: x halves
            # first so qkT can start, then the g=0 weights, then wv) ----
            xts = []
            for cc in range(CC):
                xt_t = xtp.tile([P, T], F32R, tag="xt", name=f"xt{cc}")
                nc.sync.dma_start(xt_t[:, 0:TQ], xT_r[:, cc, 0:TQ])
                xts.append(xt_t)
            for cc in range(CC):
                nc.sync.dma_start(xts[cc][:, TQ:T], xT_r[:, cc, TQ:T])

            bqk_sb = constp.tile([P, JQK], F32)
            nc.sync.dma_start(bqk_sb[:], bqk[:])
            bp_sb = constp.tile([P, EC], F32)
            nc.sync.dma_start(bp_sb[:], bp[:])
            tri_sb = constp.tile([P, 2, P], BF16)
            nc.sync.dma_start(tri_sb[:], tri[:])
            id64_sb = constp.tile([P, HD], BF16)
            nc.sync.dma_start(id64_sb[:], id64[:])

            qkT_sb = qkTp.tile([P, JQK, T], BF16)
            v_sb = vap.tile([P, NTK, G * VW + VPAD], BF16)
            v4 = v_sb[:, :, : G * VW].rearrange("p n (g w) -> p n g w", w=VW)
            rawT = rawp.tile([P, CC, T], F32R)

            # ---- v[t, j] = x Wv, interleaved per head pair ------------
            # ones columns feed the softmax-denominator trick; tail pad is
            # junk-read by the odd head's M=128 lhsT (rows 65+ of its psum)
            onec = constp.tile([P, 1], F32)
            nc.vector.memset(onec[:], 1.0)
            zeroc = constp.tile([P, 1], F32)
            nc.vector.memset(zeroc[:], 0.0)
            ones_src = onec[:, None, None, :].to_broadcast([P, NTK, G, 1])
            nc.any.tensor_copy(v4[:, :, :, HD : HD + 1], ones_src)
            nc.any.tensor_copy(v4[:, :, :, VW - 1 : VW], ones_src)
            nc.any.tensor_copy(
                v_sb[:, :, G * VW :],
                zeroc[:, None, :].to_broadcast([P, NTK, VPAD]),
            )

            wv_sb = wvp.tile([P, CC, C], F32R)

            def v_part(tcs):
                for tc_i in tcs:
                    for jn in range(C // JV):
                        ps = psB.tile([P, TQ], F32, tag="mm", name="psv")
                        for cc in range(CC):
                            nc.tensor.matmul(
                                ps[:, :JV],
                                xts[cc][:, ts(tc_i, P)],
                                wv_sb[:, cc, ts(jn, JV)],
                                start=(cc == 0),
                                stop=(cc == CC - 1),
                            )
                        g0 = jn * (JV // P)  # 3 head pairs per 384 cols
                        srcv = ps[:, :JV].rearrange(
                            "p (g h d) -> p g h d", h=2, d=HD
                        )
                        # psum -> interleaved sbuf layout on the Act engine
                        # (DVE stays free for the divide chains)
                        nc.scalar.activation(
                            v4[:, tc_i, g0 : g0 + 3, 0:HD],
                            srcv[:, :, 0, :],
                            AF.Identity,
                        )
                        nc.scalar.activation(
                            v4[:, tc_i, g0 : g0 + 3, HD + 1 : VW - 1],
                            srcv[:, :, 1, :],
                            AF.Identity,
                        )

            def load_wt(jc):
                wt = wqkp.tile([P, CC, P], F32R, tag="wqk", name="wt")
                nc.sync.dma_start(wt[:], wqk_r[:, :, ts(jc, P)])
                return wt

            def qkt_chunk(jc, wt):
                for t2 in range(NTQ):
                    ps = psB.tile([P, TQ], F32, tag="mm", name="psq")
                    for cc in range(CC):
                        nc.tensor.matmul(
                            ps[:],
                            wt[:, cc, :],
                            xts[cc][:, ts(t2, TQ)],
                            start=(cc == 0),
                            stop=(cc == CC - 1),
                        )
                    nc.scalar.activation(
                        qkT_sb[:, jc, ts(t2, TQ)],
                        ps[:],
                        AF.Identity,
                        bias=bqk_sb[:, jc : jc + 1],
                    )

            def attn_block(g, t2):
                """Emit QK/exp/AV + psum evac + denominator gather for one
                block.  Returns a closure that finishes the divide (recip,
                broadcast, multiply into rawT) — call it one block later."""
                jq, jk = g, G + g
                hi = 4 * (t2 + 1)  # causal: key chunks 0..hi-1
                avs = []
                for par in (0, 1):
                    av = psB.tile([P, TQ], F32, tag="av", name=f"av{par}")
                    avs.append(av)
                for g0 in range(0, hi, 2):
                    grp = [tkc for tkc in (g0, g0 + 1) if tkc < hi]
                    pas, css = {}, {}
                    # all QK + diag-mask matmuls of the group first: they
                    # are K=64 and pair-tile as (0,0)/(64,0); keeping them
                    # adjacent avoids PE array mode switches
                    for tkc in grp:
                        csr = tkc * P - t2 * TQ  # diag block start col
                        cs = max(0, csr)
                        pa = psA.tile([P, 2 * TQ], F32, tag="pa", name="pa")
                        pas[tkc], css[tkc] = pa, cs
                        for par in (0, 1):
                            qrow = HD * par
                            lo = cs if par == 0 else TQ
                            nc.tensor.matmul(
                                pa[:, lo : lo + TQ - cs],
                                qkT_sb[qrow : qrow + HD, jk, ts(tkc, P)],
                                qkT_sb[
                                    qrow : qrow + HD,
                                    jq,
                                    t2 * TQ + cs : (t2 + 1) * TQ,
                                ],
                                start=True,
                                stop=(csr < 0),
                            )
                        if csr >= 0:
                            # add -1e30 above the diagonal with two K=64
                            # triangular matmuls (same row-tiled mode):
                            # pa[tk, lo+j] += tri[j, tk]
                            for par in (0, 1):
                                qrow = HD * par
                                lo = cs if par == 0 else TQ
                                for half in (0, 1):
                                    nc.tensor.matmul(
                                        pa[
                                            :,
                                            lo + HD * half : lo + HD * (half + 1),
                                        ],
                                        tri_sb[qrow : qrow + HD, half, :],
                                        id64_sb[qrow : qrow + HD, :],
                                        start=False,
                                        stop=(half == 1),
                                    )
                    for tkc in grp:
                        pa, cs = pas[tkc], css[tkc]
                        e = expp.tile([P, 2 * TQ], BF16, tag="exp", name="e")
                        nc.scalar.activation(
                            e[:, cs : 2 * TQ - cs],
                            pa[:, cs : 2 * TQ - cs],
                            AF.Exp,
                            scale=SCALE,
                        )
                        for par in (0, 1):
                            lo = cs if par == 0 else TQ
                            vlo = g * VW + (HD + 1) * par
                            nc.tensor.matmul(
                                avs[par][:, cs:],
                                v_sb[:, tkc, vlo : vlo + P],
                                e[:, lo : lo + TQ - cs],
                                start=(tkc == 0),
                                stop=(tkc == hi - 1),
                            )
                # evacuate out_aug to SBUF right away in bf16 (frees the
                # psum banks fast) and gather the two denominator rows to
                # [128, 8] via DMA so the reciprocal runs on all lanes.
                asb = rrp.tile([65, 2, TQ], BF16, tag="avsb", name="asb")
                for par in (0, 1):
                    nc.vector.tensor_scalar_add(
                        asb[:, par, :], avs[par][0:65, :], 0.0
                    )
                rd = rrp.tile([P, 8], BF16, tag="rd", name="rd")
                nc.sync.dma_start(rd[:], asb[64:65, :, :])

                def finish():
                    rd2 = rrp.tile([P, 8], BF16, tag="rd2", name="rd2")
                    with nc.allow_low_precision(
                        reason="softmax denom reciprocal in bf16; rel err "
                        "~4e-3 well inside the 2e-2 budget"
                    ):
                        nc.vector.reciprocal(rd2[:], rd[:])
                    rro = rrp.tile([1, 2, TQ], BF16, tag="rro", name="rro")
                    nc.sync.dma_start(rro[0:1, :, :], rd2[:])
                    rrs = []
                    for par in (0, 1):
                        rr = rrp.tile([64, TQ], BF16, tag=f"rr{par}", name="rr")
                        nc.sync.dma_start(
                            rr[:],
                            rro[0:1, par, None, :].to_broadcast([1, 64, TQ]),
                        )
                        rrs.append(rr)
                    nc.vector.tensor_tensor(
                        rawT[0:64, g, ts(t2, TQ)],
                        asb[0:64, 0, :],
                        rrs[0][:],
                        MUL,
                    )
                    tmp = rrp.tile([64, TQ], F32R, tag="otmp", name="tmp")
                    nc.vector.tensor_tensor(
                        tmp[:], asb[0:64, 1, :], rrs[1][:], MUL
                    )
                    nc.sync.dma_start(rawT[64:128, g, ts(t2, TQ)], tmp[:])

                return finish

            def load_wpt(ec):
                wpt = wpp.tile([P, CC, P], F32R, tag="wp", name="wpt")
                nc.sync.dma_start(wpt[:], wp_r[:, :, ts(ec, P)])
                return wpt

            def proj_unit(t2, ec, wpt):
                ps = psB.tile([P, TQ], F32, tag="mm", name="psp_")
                for jc in range(CC):
                    nc.tensor.matmul(
                        ps[:],
                        wpt[:, jc, :],
                        rawT[:, jc, ts(t2, TQ)],
                        start=(jc == 0),
                        stop=(jc == CC - 1),
                    )
                yt = ytp.tile([P, TQ], F32, tag="yt", name="yt")
                nc.vector.tensor_scalar_add(yt[:], ps[:], bp_sb[:, ec : ec + 1])
                nc.sync.dma_start(yT_r[:, ec, ts(t2, TQ)], yt[:])

            def proj_partial(ec, wpt):
                # t2=1, first 5 head-pair chunks; evac partial on Act
                ps = psB.tile([P, TQ], F32, tag="mm", name="pspa")
                for jc in range(CC - 1):
                    nc.tensor.matmul(
                        ps[:],
                        wpt[:, jc, :],
                        rawT[:, jc, ts(1, TQ)],
                        start=(jc == 0),
                        stop=(jc == CC - 2),
                    )
                yta = ytap.tile([P, TQ], F32, tag="yta", name="yta")
                nc.scalar.activation(yta[:], ps[:], AF.Identity)
                return yta

            def proj_final(ec, wpt, yta):
                ps = psB.tile([P, TQ], F32, tag="mm", name="pspb")
                nc.tensor.matmul(
                    ps[:],
                    wpt[:, CC - 1, :],
                    rawT[:, CC - 1, ts(1, TQ)],
                    start=True,
                    stop=True,
                )
                yt = ytp.tile([P, TQ], F32, tag="yt", name="yt")
                nc.vector.scalar_tensor_tensor(
                    yt[:], ps[:], bp_sb[:, ec : ec + 1], yta[:], ADD, ADD
                )
                nc.sync.dma_start(yT_r[:, ec, ts(1, TQ)], yt[:])

            # ---- emission schedule ------------------------------------
            wts = (load_wt(0), load_wt(G))
            nc.sync.dma_start(wv_sb[:], wv_r)
            qkt_chunk(0, wts[0])
            qkt_chunk(G, wts[1])
            v_part(range(0, 4))
            nxt = (load_wt(1), load_wt(G + 1))
            fin = attn_block(0, 0)
            v_part(range(4, 6))
            wpts = []
            for g in range(1, G):
                wts = nxt
                qkt_chunk(g, wts[0])
                qkt_chunk(G + g, wts[1])
                if g < G - 1:
                    nxt = (load_wt(g + 1), load_wt(G + g + 1))
                else:
                    wpts = [load_wpt(ec) for ec in range(EC)]
                if g == 1:
                    v_part(range(6, 8))
                prev, fin = fin, attn_block(g, 0)
                prev()
            for g in range(G):
                prev, fin = fin, attn_block(g, 1)
                prev()
                if g >= 1:
                    proj_unit(0, g - 1, wpts[g - 1])
            fin()  # divide for the last block (5, 1)
            proj_unit(0, EC - 1, wpts[EC - 1])
            ytas = [proj_partial(ec, wpts[ec]) for ec in range(EC)]
            for ec in range(EC):
                proj_final(ec, wpts[ec], ytas[ec])

    nc.compile()
    _CACHE["nc"] = nc
    return nc


def _round_fp32r(a):
    """Round fp32 to fp32r (11-bit mantissa) the way the PE expects."""
    u = np.ascontiguousarray(a, dtype=np.float32).view(np.uint32)
    u = ((u.astype(np.uint64) + 0x800) & 0xFFFFF000).astype(np.uint32)
    return u.view(np.float32)


def make_in_maps(x, w_attn, b_attn, w_proj, b_proj):
    x = np.ascontiguousarray(np.asarray(x, dtype=np.float32))
    w_attn = np.ascontiguousarray(np.asarray(w_attn, dtype=np.float32))
    b_attn = np.ascontiguousarray(np.asarray(b_attn, dtype=np.float32))
    w_proj = np.ascontiguousarray(np.asarray(w_proj, dtype=np.float32))
    b_proj = np.ascontiguousarray(np.asarray(b_proj, dtype=np.float32))

    bf = ml_dtypes.bfloat16
    wqk = _round_fp32r(w_attn[:, : 2 * C])
    wv = _round_fp32r(w_attn[:, 2 * C :])
    w_proj_r = _round_fp32r(w_proj)
    bqk = np.ascontiguousarray(b_attn[: 2 * C].reshape(JQK, P).T)
    # the v bias folds into the projection bias: y = Wp.T (raw + bv) + bp
    bv = b_attn[2 * C :].astype(np.float64)
    bp_eff = (w_proj.astype(np.float64).T @ bv + b_proj).astype(np.float32)
    bp = np.ascontiguousarray(bp_eff.reshape(EC, P).T)
    # tri[qrow+r, 0, tk] masks j=r    < tk; tri[qrow+r, 1, tk] masks 64+r < tk
    tri = np.zeros((P, 2, P), dtype=bf)
    tk = np.arange(P)[None, :]
    for qrow in (0, HD):
        r = np.arange(HD)[:, None]
        tri[qrow : qrow + HD, 0, :] = np.where(r < tk, -1e30, 0.0).astype(bf)
        tri[qrow : qrow + HD, 1, :] = np.where(r + HD < tk, -1e30, 0.0).astype(
            bf
        )
    id64 = np.zeros((P, HD), dtype=bf)
    for qrow in (0, HD):
        id64[qrow : qrow + HD, :] = np.eye(HD, dtype=bf)

    shared = {
        "wqk": wqk,
        "wv": wv,
        "wp": w_proj_r,
        "bqk": bqk,
        "bp": bp,
        "tri": tri,
        "id64": id64,
    }
    return [
        {"xT": _round_fp32r(x[b].T), **shared} for b in range(NCORES)
    ]


def kernel(**inputs):
    nc = _build()
    in_maps = make_in_maps(
        inputs["x"],
        inputs["w_attn"],
        inputs["b_attn"],
        inputs["w_proj"],
        inputs["b_proj"],
    )
    res = run_bass_kernel_spmd(nc, in_maps, list(range(NCORES)))
    out = np.stack(
        [np.ascontiguousarray(res.results[b]["yT"].T) for b in range(NCORES)]
    )
    return out.astype(np.float32)
